# revision 1
# baseline (speedup 1.0000x reference)
"""Trainium2 Bass kernel for CrossAttention.

  y = softmax((x@Wq) @ (ctx@Wk)^T / sqrt(D)) @ (ctx@Wv) @ Wo + bo

Shapes: x [16, 4096, 1024], context [16, 77, 768], H=8 heads, D=64.
Sharding: pure data-parallel over batch B — each of the 8 cores gets 2
batches; no collectives.

Per-core device program (all matmuls bf16, fp32 PSUM accumulation):
  - host pre-transposes x -> xT [2, 1024, 4096] bf16 so the contraction
    dim lands on SBUF partitions without any on-device bulk transpose.
  - qT[c] = Wq_chunk^T-free matmul: psum[128 inner, 512 tok]
  - scoresT_h [77, tok] = kT_h^T-contraction matmul (K=64)
  - exp on ScalarE (PSUM->SBUF, bf16); per-head attention sums come for
    free from a ones-column appended to v (v_aug [77, 65]).
  - o natural [tok, 64] per head via lhsT=expT chunk; normalization is a
    per-partition reciprocal + one broadcast tensor_tensor multiply.
  - o transposed back (PE transpose) to feed y = o @ Wo + bo; bias is a
    K=1 ones-row matmul accumulated into the same PSUM group.
"""

import os

import numpy as np
import ml_dtypes

import bass_rust
import concourse.bass as bass
import concourse.mybir as mybir
import concourse.tile as _tile
from concourse.bass_utils import run_bass_kernel_spmd
from concourse.masks import make_identity
from concourse.vector_clock import ScopedClock

# ---------------------------------------------------------------------------
# Workaround: this walrus build rejects >1 sem-wait on one SP CTRL
# instruction ("Too many sync wait commands").  Split the Tile tail-drain
# waits across multiple Drain instructions (one wait each).
_MAXW = 1


def _split_drain_and_barrier(self, tick_clock, wait_clock):
    nc = self.nc
    drain_inst = nc.sync.drain()
    wait_clock.add_sem_waits(
        drain_inst.ins, ScopedClock({None: tick_clock.global_clock})
    )
    si = drain_inst.ins.sync_info
    if si is not None and len(si.on_wait) > _MAXW:
        waits = list(si.on_wait)
        upd = list(si.on_update)
        drain_inst.ins.sync_info = bass_rust.SyncInfo(
            on_wait=waits[:_MAXW], on_update=upd
        )
        for i in range(_MAXW, len(waits), _MAXW):
            extra = nc.sync.drain()
            extra.ins.sync_info = bass_rust.SyncInfo(
                on_wait=waits[i : i + _MAXW], on_update=[]
            )
    nc.all_engine_barrier()
    assert self.sems is not None
    popped = nc._tile_sem_poison_stack.pop()
    assert popped is self._sem_poison
    nc.clear_and_free_semaphores(list(self.sems.allocated().values()))
    nc.all_engine_barrier()


_tile.TileContext._drain_and_barrier = _split_drain_and_barrier

_ws_counter = [0]


def _split_excess_waits(nc, maxw=_MAXW):
    """Walrus here accepts only `maxw` sem-waits per instruction; move the
    excess onto preceding same-engine NoOps (identical blocking semantics)."""
    for fn in nc.m.functions:
        for bb in fn.blocks:
            new = []
            for inst in bb.instructions:
                si = inst.sync_info
                if si is not None and len(si.on_wait) > maxw:
                    waits = list(si.on_wait)
                    upd = list(si.on_update)
                    extra, keep = waits[:-maxw], waits[-maxw:]
                    for i in range(0, len(extra), maxw):
                        nop = mybir.InstNoOp(
                            name=f"waitsplit-{_ws_counter[0]}", ins=[], outs=[]
                        )
                        _ws_counter[0] += 1
                        nop.engine = inst.engine
                        nop.sync_info = bass_rust.SyncInfo(
                            on_wait=extra[i : i + maxw], on_update=[]
                        )
                        new.append(nop)
                    inst.sync_info = bass_rust.SyncInfo(
                        on_wait=keep, on_update=upd
                    )
                new.append(inst)
            bb.instructions = new

# ---------------------------------------------------------------------------
# Problem constants (hardcoded per contract)
B, N, M = 16, 4096, 77
Q_DIM, C_DIM = 1024, 768
H, D = 8, 64
INNER = H * D  # 512
N_CORES = 8
B_LOC = B // N_CORES  # 2 batches per core

P = 128
KQ = Q_DIM // P  # 8 feature chunks of x
KC = C_DIM // P  # 6 feature chunks of context
IC = INNER // P  # 4 inner chunks
TQ = 512  # tokens per macro-tile
NT = N // TQ  # 8 macro-tiles per batch
TC = TQ // P  # 4 token chunks of 128 inside a macro-tile

BF16 = mybir.dt.bfloat16
F32 = mybir.dt.float32

LAST_RESULTS = None  # BassKernelResults of the most recent run (for test.py)


def _build_program():
    nc = bass.Bass()
    xT = nc.dram_tensor("xT", [B_LOC, Q_DIM, N], BF16, kind="ExternalInput")
    ctxT = nc.dram_tensor("ctxT", [B_LOC, C_DIM, M], BF16, kind="ExternalInput")
    wq = nc.dram_tensor("wq", [Q_DIM, INNER], BF16, kind="ExternalInput")
    wk = nc.dram_tensor("wk", [C_DIM, INNER], BF16, kind="ExternalInput")
    wv = nc.dram_tensor("wv", [C_DIM, INNER], BF16, kind="ExternalInput")
    wo = nc.dram_tensor("wo", [INNER, Q_DIM], BF16, kind="ExternalInput")
    bo = nc.dram_tensor("bo", [P, Q_DIM], BF16, kind="ExternalInput")
    y = nc.dram_tensor("y", [B_LOC * N, Q_DIM], F32, kind="ExternalOutput")

    with _tile.TileContext(nc) as tc:
        with (
            tc.tile_pool(name="const", bufs=1) as const,
            tc.tile_pool(name="kv", bufs=2) as kvp,
            tc.tile_pool(name="kt", bufs=18) as ktp,
            tc.tile_pool(name="xin", bufs=2) as xp,
            tc.tile_pool(name="qt", bufs=10) as qp,
            tc.tile_pool(name="st", bufs=10) as sp,
            tc.tile_pool(name="ot", bufs=4) as op_,
            tc.tile_pool(name="yo", bufs=2) as yp,
            tc.tile_pool(name="ps_qy", bufs=2, space="PSUM") as ps_qy,
            tc.tile_pool(name="ps_s", bufs=2, space="PSUM") as ps_s,
            tc.tile_pool(name="ps_o", bufs=2, space="PSUM") as ps_o,
            tc.tile_pool(name="ps_t", bufs=2, space="PSUM") as ps_t,
        ):
            # ---- constants / weights ----
            wq_sb = const.tile([P, KQ, INNER], BF16)
            nc.sync.dma_start(out=wq_sb[:], in_=wq.rearrange("(k p) i -> p k i", p=P))
            wk_sb = const.tile([P, KC, INNER], BF16)
            nc.sync.dma_start(out=wk_sb[:], in_=wk.rearrange("(k p) i -> p k i", p=P))
            wv_sb = const.tile([P, KC, INNER], BF16)
            nc.sync.dma_start(out=wv_sb[:], in_=wv.rearrange("(k p) i -> p k i", p=P))
            wo_sb = const.tile([P, IC, Q_DIM], BF16)
            nc.sync.dma_start(out=wo_sb[:], in_=wo.rearrange("(c p) o -> p c o", p=P))
            bo_bc = const.tile([P, Q_DIM], BF16)
            nc.sync.dma_start(out=bo_bc[:], in_=bo[:, :])
            ident = const.tile([P, P], BF16)
            make_identity(nc, ident[:])

            for b in range(B_LOC):
                # ---- per-batch K^T and V(+ones) ----
                ctx_sb = kvp.tile([P, KC, M], BF16, tag="ctx")
                nc.sync.dma_start(
                    out=ctx_sb[:], in_=ctxT[b].rearrange("(k p) m -> p k m", p=P)
                )
                kts = []  # IC head-pair tiles [128, M]; head 2c rows 0:64, 2c+1 rows 64:128
                for c in range(IC):
                    pk = ps_s.tile([P, M], F32, tag="s")
                    for f in range(KC):
                        nc.tensor.matmul(
                            pk[:],
                            lhsT=wk_sb[:, f, c * P : (c + 1) * P],
                            rhs=ctx_sb[:, f, :],
                            start=(f == 0),
                            stop=(f == KC - 1),
                        )
                    kt = ktp.tile([P, M], BF16, tag="kt")
                    nc.vector.tensor_copy(kt[:], pk[:])
                    kts.append(kt)
                va = kvp.tile([M, H, 65], BF16, tag="va")
                pv = ps_s.tile([M, INNER], F32, tag="s")
                for f in range(KC):
                    nc.tensor.matmul(
                        pv[:],
                        lhsT=ctx_sb[:, f, :],
                        rhs=wv_sb[:, f, :],
                        start=(f == 0),
                        stop=(f == KC - 1),
                    )
                nc.vector.tensor_copy(
                    va[:, :, 0:64], pv.rearrange("p (h d) -> p h d", h=H)
                )
                nc.vector.memset(va[:, :, 64:65], 1.0)

                for t in range(NT):
                    t0 = t * TQ
                    # ---- load x^T macro-tile ----
                    xt = xp.tile([P, KQ, TQ], BF16, tag="x")
                    nc.sync.dma_start(
                        out=xt[:],
                        in_=xT[b].rearrange("(k p) t -> p k t", p=P)[
                            :, :, t0 : t0 + TQ
                        ],
                    )
                    # ---- q^T (head-pair chunk tiles) ----
                    qts = []
                    for c in range(IC):
                        pq = ps_qy.tile([P, TQ], F32, tag="qy")
                        for k in range(KQ):
                            nc.tensor.matmul(
                                pq[:],
                                lhsT=wq_sb[:, k, c * P : (c + 1) * P],
                                rhs=xt[:, k, :],
                                start=(k == 0),
                                stop=(k == KQ - 1),
                            )
                        qt = qp.tile([P, TQ], BF16, tag="qt")
                        nc.scalar.copy(qt[:], pq[:])
                        qts.append(qt)
                    # ---- scores^T + exp (head pairs packed on PE rows) ----
                    sts = []
                    for h in range(H):
                        c, hh = h // 2, h % 2
                        rows = slice(hh * 64, (hh + 1) * 64)
                        ps = ps_s.tile([M, TQ], F32, tag="s")
                        nc.tensor.matmul(
                            ps[:],
                            lhsT=kts[c][rows, :],
                            rhs=qts[c][rows, :],
                            start=True,
                            stop=True,
                        )
                        st = sp.tile([M, TQ], BF16, tag="st")
                        nc.scalar.activation(
                            st[:], ps[:], mybir.ActivationFunctionType.Exp
                        )
                        sts.append(st)
                    # ---- per token-chunk: PV, normalize, transpose, y ----
                    for tcc in range(TC):
                        tok = slice(tcc * P, (tcc + 1) * P)
                        o_sb = op_.tile([P, INNER], BF16, tag="o")
                        o_v = o_sb.rearrange("p (h d) -> p h d", d=64)
                        rec = op_.tile([P, H, 1], F32, tag="rec")
                        for g in range(2):
                            po = ps_o.tile([P, 4 * 65], F32, tag="o")
                            for j in range(4):
                                h = g * 4 + j
                                nc.tensor.matmul(
                                    po[:, j * 65 : (j + 1) * 65],
                                    lhsT=sts[h][:, tok],
                                    rhs=va[:, h, :],
                                    start=True,
                                    stop=True,
                                )
                            pov = po.rearrange("p (h x) -> p h x", x=65)
                            nc.vector.reciprocal(
                                rec[:, g * 4 : (g + 1) * 4, :], pov[:, :, 64:65]
                            )
                            nc.vector.tensor_tensor(
                                out=o_v[:, g * 4 : (g + 1) * 4, :],
                                in0=pov[:, :, 0:64],
                                in1=rec.rearrange("p h x -> p (h x)")[
                                    :, g * 4 : (g + 1) * 4
                                ].broadcast_to([P, 4, 64]),
                                op=mybir.AluOpType.mult,
                            )
                        ot = op_.tile([P, IC, P], BF16, tag="ot")
                        for icc in range(IC):
                            pt = ps_t.tile([P, P], BF16, tag="t")
                            nc.tensor.transpose(
                                pt[:], o_sb[:, icc * P : (icc + 1) * P], ident[:]
                            )
                            nc.scalar.copy(ot[:, icc, :], pt[:])
                        ysb = yp.tile([P, Q_DIM], F32, tag="y")
                        for half in range(2):
                            col = slice(half * 512, (half + 1) * 512)
                            py = ps_qy.tile([P, 512], F32, tag="qy")
                            for icc in range(IC):
                                nc.tensor.matmul(
                                    py[:],
                                    lhsT=ot[:, icc, :],
                                    rhs=wo_sb[:, icc, col],
                                    start=(icc == 0),
                                    stop=(icc == IC - 1),
                                )
                            nc.vector.tensor_tensor(
                                out=ysb[:, col],
                                in0=py[:],
                                in1=bo_bc[:, col],
                                op=mybir.AluOpType.add,
                            )
                        row0 = b * N + t0 + tcc * P
                        nc.sync.dma_start(out=y[row0 : row0 + P, :], in_=ysb[:])
                # rotate per-batch kt list (kts rebuilt next batch)
    _split_excess_waits(nc)
    return nc


def prep_in_maps(x, context, Wq, Wk, Wv, Wo, bo):
    bf = ml_dtypes.bfloat16
    # host-side prep: bf16 cast + pre-transpose so contraction dims are
    # contiguous on device partitions
    xT = np.ascontiguousarray(
        np.asarray(x, dtype=np.float32).transpose(0, 2, 1)
    ).astype(bf)
    ctxT = np.ascontiguousarray(
        np.asarray(context, dtype=np.float32).transpose(0, 2, 1)
    ).astype(bf)
    scale = np.float32(1.0 / np.sqrt(D))
    wq_h = (np.asarray(Wq, dtype=np.float32) * scale).astype(bf)
    wk_h = np.asarray(Wk, dtype=np.float32).astype(bf)
    wv_h = np.asarray(Wv, dtype=np.float32).astype(bf)
    wo_h = np.asarray(Wo, dtype=np.float32).astype(bf)
    bo_h = np.ascontiguousarray(
        np.broadcast_to(
            np.asarray(bo, dtype=np.float32).reshape(1, Q_DIM), (P, Q_DIM)
        )
    ).astype(bf)
    in_maps = []
    for c in range(N_CORES):
        in_maps.append(
            {
                "xT": xT[c * B_LOC : (c + 1) * B_LOC],
                "ctxT": ctxT[c * B_LOC : (c + 1) * B_LOC],
                "wq": wq_h,
                "wk": wk_h,
                "wv": wv_h,
                "wo": wo_h,
                "bo": bo_h,
            }
        )
    return in_maps


def kernel(x, context, Wq, Wk, Wv, Wo, bo):
    global LAST_RESULTS
    in_maps = prep_in_maps(x, context, Wq, Wk, Wv, Wo, bo)
    nc = _build_program()
    trace = bool(int(os.environ.get("BASS_KERNEL_TRACE", "0")))
    res = run_bass_kernel_spmd(
        nc, in_maps, core_ids=list(range(N_CORES)), trace=trace
    )
    LAST_RESULTS = res
    out = np.empty((B, N, Q_DIM), dtype=np.float32)
    for c in range(N_CORES):
        out[c * B_LOC : (c + 1) * B_LOC] = res.results[c]["y"].reshape(
            B_LOC, N, Q_DIM
        )
    return out



# revision 12
# speedup vs baseline: 7.4131x; 7.4131x over previous
"""Trainium2 Bass kernel for CrossAttention.

  y = softmax((x@Wq) @ (ctx@Wk)^T / sqrt(D)) @ (ctx@Wv) @ Wo + bo

Shapes: x [16, 4096, 1024], context [16, 77, 768], H=8 heads, D=64.
Sharding: pure data-parallel over batch B — each of the 8 cores gets 2
batches; no collectives.

Per-core device program (all matmuls bf16, fp32 PSUM accumulation),
software-pipelined with a one-macro-tile skew so the PE never waits on
the Act/DVE round-trips (qt copy, exp, normalize):

  step g: [Qproj chunks + scores/exp of tile g] interleaved with
          [PV + transpose + Yproj + store of tile g-1]

  - host pre-transposes x -> xT [2, 1024, 4096] bf16 so the contraction
    dim lands on SBUF partitions without any on-device bulk transpose.
  - qT[c] chunks [128 inner, 512 tok] from a shared PSUM ring.
  - scoresT_h [77, tok] matmul (K=64) emitted per Q-chunk pair; exp on
    ScalarE (PSUM->SBUF, bf16); per-head attention sums come free from a
    ones-column appended to v (v_aug [77, 65]).
  - o natural [tok, 64] per head via lhsT=expT chunk; normalization is a
    per-partition reciprocal + broadcast tensor_tensor multiply (DVE).
  - per token-chunk all 4 PE transposes write one packed PSUM bank
    [128, 4, 128] -> single ot copy; y = o @ Wo, bias added on DVE.
PSUM budget: ring(4) + po(2) + py(2) = 8 banks.
"""

import os

import numpy as np
import ml_dtypes

import bass_rust
import concourse.bass as bass
import concourse.mybir as mybir
import concourse.tile as _tile
from concourse.bass_utils import run_bass_kernel_spmd
from concourse.masks import make_identity
from concourse.vector_clock import ScopedClock

# ---------------------------------------------------------------------------
# Workaround: this walrus build rejects >1 sem-wait on one SP CTRL
# instruction ("Too many sync wait commands").  Split the Tile tail-drain
# waits across multiple Drain instructions (one wait each).
_MAXW = 1


def _split_drain_and_barrier(self, tick_clock, wait_clock):
    nc = self.nc
    drain_inst = nc.sync.drain()
    wait_clock.add_sem_waits(
        drain_inst.ins, ScopedClock({None: tick_clock.global_clock})
    )
    si = drain_inst.ins.sync_info
    if si is not None and len(si.on_wait) > _MAXW:
        waits = list(si.on_wait)
        upd = list(si.on_update)
        drain_inst.ins.sync_info = bass_rust.SyncInfo(
            on_wait=waits[:_MAXW], on_update=upd
        )
        for i in range(_MAXW, len(waits), _MAXW):
            extra = nc.sync.drain()
            extra.ins.sync_info = bass_rust.SyncInfo(
                on_wait=waits[i : i + _MAXW], on_update=[]
            )
    nc.all_engine_barrier()
    assert self.sems is not None
    popped = nc._tile_sem_poison_stack.pop()
    assert popped is self._sem_poison
    nc.clear_and_free_semaphores(list(self.sems.allocated().values()))
    nc.all_engine_barrier()


_tile.TileContext._drain_and_barrier = _split_drain_and_barrier

_ws_counter = [0]


def _split_excess_waits(nc, maxw=_MAXW):
    """Walrus here accepts only `maxw` sem-waits per instruction; move the
    excess onto preceding same-engine NoOps (identical blocking semantics)."""
    for fn in nc.m.functions:
        for bb in fn.blocks:
            new = []
            for inst in bb.instructions:
                si = inst.sync_info
                if si is not None and len(si.on_wait) > maxw:
                    waits = list(si.on_wait)
                    upd = list(si.on_update)
                    extra, keep = waits[:-maxw], waits[-maxw:]
                    for i in range(0, len(extra), maxw):
                        nop = mybir.InstNoOp(
                            name=f"waitsplit-{_ws_counter[0]}", ins=[], outs=[]
                        )
                        _ws_counter[0] += 1
                        nop.engine = inst.engine
                        nop.sync_info = bass_rust.SyncInfo(
                            on_wait=extra[i : i + maxw], on_update=[]
                        )
                        new.append(nop)
                    inst.sync_info = bass_rust.SyncInfo(
                        on_wait=keep, on_update=upd
                    )
                new.append(inst)
            bb.instructions = new

# ---------------------------------------------------------------------------
# Problem constants (hardcoded per contract)
B, N, M = 16, 4096, 77
Q_DIM, C_DIM = 1024, 768
H, D = 8, 64
INNER = H * D  # 512
N_CORES = 8
B_LOC = B // N_CORES  # 2 batches per core

P = 128
KQ = Q_DIM // P  # 8 feature chunks of x
KC = C_DIM // P  # 6 feature chunks of context
IC = INNER // P  # 4 inner chunks
TQ = 512  # tokens per macro-tile
NT = N // TQ  # 8 macro-tiles per batch
TC = TQ // P  # 4 token chunks of 128 inside a macro-tile
G = B_LOC * NT  # 16 macro-tile steps per core

BF16 = mybir.dt.bfloat16
F32 = mybir.dt.float32

LAST_RESULTS = None  # BassKernelResults of the most recent run (for test.py)


def _build_program():
    nc = bass.Bass()
    xT = nc.dram_tensor("xT", [B_LOC, Q_DIM, N], BF16, kind="ExternalInput")
    ctxT = nc.dram_tensor("ctxT", [B_LOC, C_DIM, M], BF16, kind="ExternalInput")
    wq = nc.dram_tensor("wq", [Q_DIM, INNER], BF16, kind="ExternalInput")
    wk = nc.dram_tensor("wk", [C_DIM, INNER], BF16, kind="ExternalInput")
    wv = nc.dram_tensor("wv", [C_DIM, INNER], BF16, kind="ExternalInput")
    wo = nc.dram_tensor("wo", [INNER, Q_DIM], BF16, kind="ExternalInput")
    bo = nc.dram_tensor("bo", [P, Q_DIM], BF16, kind="ExternalInput")
    y = nc.dram_tensor("y", [B_LOC * N, Q_DIM], F32, kind="ExternalOutput")

    with _tile.TileContext(nc) as tc:
        with (
            tc.tile_pool(name="const", bufs=1) as const,
            tc.tile_pool(name="kv", bufs=2) as kvp,
            tc.tile_pool(name="kt", bufs=8) as ktp,
            tc.tile_pool(name="xin", bufs=2) as xp,
            tc.tile_pool(name="qt", bufs=2) as qp,
            tc.tile_pool(name="st", bufs=2) as sp,
            tc.tile_pool(name="ot", bufs=2) as op_,
            tc.tile_pool(name="yo", bufs=2) as yp,
            tc.tile_pool(name="ring", bufs=4, space="PSUM") as ring,
            tc.tile_pool(name="ps_o", bufs=2, space="PSUM") as ps_o,
            tc.tile_pool(name="ps_y", bufs=2, space="PSUM") as ps_y,
        ):
            # ---- constants / weights ----
            # DMA order matters for pipeline fill: wq + the first x tile
            # first (they gate the first Q-proj matmuls), then the K/V-proj
            # weights, then wo/bo (not needed until step 1).
            wq_sb = const.tile([P, KQ, INNER], BF16)
            nc.sync.dma_start(out=wq_sb[:], in_=wq.rearrange("(k p) i -> p k i", p=P))
            ident = const.tile([P, P], BF16)
            make_identity(nc, ident[:])

            # pipeline state
            kv_state = {}  # b -> (kts, va)
            xt_t = {}      # g -> xt tile
            qt_t = {}      # g -> [4 qt tiles]
            st_t = {}      # g -> [8 st tiles]
            late_consts = {}

            def emit_kv_weights():
                wk_sb = const.tile([P, KC, INNER], BF16)
                nc.sync.dma_start(
                    out=wk_sb[:], in_=wk.rearrange("(k p) i -> p k i", p=P)
                )
                wv_sb = const.tile([P, KC, INNER], BF16)
                nc.sync.dma_start(
                    out=wv_sb[:], in_=wv.rearrange("(k p) i -> p k i", p=P)
                )
                late_consts.update(wk_sb=wk_sb, wv_sb=wv_sb)

            def emit_out_weights():
                wo_sb = const.tile([P, IC, Q_DIM], BF16)
                nc.sync.dma_start(
                    out=wo_sb[:], in_=wo.rearrange("(c p) o -> p c o", p=P)
                )
                bo_bc = const.tile([P, Q_DIM], BF16)
                nc.sync.dma_start(out=bo_bc[:], in_=bo[:, :])
                late_consts.update(wo_sb=wo_sb, bo_bc=bo_bc)

            def emit_kv(b):
                wk_sb, wv_sb = late_consts["wk_sb"], late_consts["wv_sb"]
                ctx_sb = kvp.tile([P, KC, M], BF16, tag="ctx")
                nc.sync.dma_start(
                    out=ctx_sb[:], in_=ctxT[b].rearrange("(k p) m -> p k m", p=P)
                )
                kts = []
                for c in range(IC):
                    pk = ring.tile([P, TQ], F32, tag="r")
                    for f in range(KC):
                        nc.tensor.matmul(
                            pk[:, 0:M],
                            lhsT=wk_sb[:, f, c * P : (c + 1) * P],
                            rhs=ctx_sb[:, f, :],
                            start=(f == 0),
                            stop=(f == KC - 1),
                        )
                    kt = ktp.tile([P, M], BF16, tag="kt")
                    nc.vector.tensor_copy(kt[:], pk[:, 0:M])
                    kts.append(kt)
                va = kvp.tile([M, H, 65], BF16, tag="va")
                pv = ring.tile([P, TQ], F32, tag="r")
                for f in range(KC):
                    nc.tensor.matmul(
                        pv[0:M, :],
                        lhsT=ctx_sb[:, f, :],
                        rhs=wv_sb[:, f, :],
                        start=(f == 0),
                        stop=(f == KC - 1),
                    )
                nc.vector.tensor_copy(
                    va[:, :, 0:64], pv[0:M, :].rearrange("p (h d) -> p h d", h=H)
                )
                nc.vector.memset(va[:, :, 64:65], 1.0)
                kv_state[b] = (kts, va)

            def emit_dma_x(g):
                b, t = g // NT, g % NT
                t0 = t * TQ
                xt = xp.tile([P, KQ, TQ], BF16, tag="x")
                nc.sync.dma_start(
                    out=xt[:],
                    in_=xT[b].rearrange("(k p) t -> p k t", p=P)[
                        :, :, t0 : t0 + TQ
                    ],
                )
                xt_t[g] = xt

            def emit_qchunk(g, c):
                xt = xt_t[g]
                pq = ring.tile([P, TQ], F32, tag="r")
                for k in range(KQ):
                    nc.tensor.matmul(
                        pq[:],
                        lhsT=wq_sb[:, k, c * P : (c + 1) * P],
                        rhs=xt[:, k, :],
                        start=(k == 0),
                        stop=(k == KQ - 1),
                    )
                qt = qp.tile([P, TQ], BF16, tag=f"qt{c}")
                nc.scalar.copy(qt[:], pq[:])
                qt_t.setdefault(g, {})[c] = qt

            def emit_score(g, h):
                b = g // NT
                kts, _ = kv_state[b]
                c, hh = h // 2, h % 2
                rows = slice(hh * 64, (hh + 1) * 64)
                ps = ring.tile([P, TQ], F32, tag="r")
                nc.tensor.matmul(
                    ps[0:M, :],
                    lhsT=kts[c][rows, :],
                    rhs=qt_t[g][c][rows, :],
                    start=True,
                    stop=True,
                )
                st = sp.tile([M, TQ], BF16, tag=f"st{h}")
                nc.scalar.activation(
                    st[:], ps[0:M, :], mybir.ActivationFunctionType.Exp
                )
                st_t.setdefault(g, {})[h] = st

            def emit_pv(g, tcc):
                b = g // NT
                _, va = kv_state[b]
                sts = st_t[g]
                tok = slice(tcc * P, (tcc + 1) * P)
                o_sb = op_.tile([P, INNER], BF16, tag="o")
                o_v = o_sb.rearrange("p (h d) -> p h d", d=64)
                rec = op_.tile([P, H, 1], F32, tag="rec")
                for gg in range(2):
                    po = ps_o.tile([P, 4 * 65], F32, tag="o")
                    for j in range(4):
                        h = gg * 4 + j
                        nc.tensor.matmul(
                            po[:, j * 65 : (j + 1) * 65],
                            lhsT=sts[h][:, tok],
                            rhs=va[:, h, :],
                            start=True,
                            stop=True,
                        )
                    pov = po.rearrange("p (h x) -> p h x", x=65)
                    nc.vector.reciprocal(
                        rec[:, gg * 4 : (gg + 1) * 4, :], pov[:, :, 64:65]
                    )
                    nc.vector.tensor_tensor(
                        out=o_v[:, gg * 4 : (gg + 1) * 4, :],
                        in0=pov[:, :, 0:64],
                        in1=rec.rearrange("p h x -> p (h x)")[
                            :, gg * 4 : (gg + 1) * 4
                        ].broadcast_to([P, 4, 64]),
                        op=mybir.AluOpType.mult,
                    )
                return o_sb

            o_state = {}  # (g, tcc) -> o_sb
            ot_state = {}  # (g, tcc) -> ot

            def emit_tr(g, tcc):
                o_sb = o_state.pop((g, tcc))
                pt = ring.tile([P, IC, P], BF16, tag="r")
                for icc in range(IC):
                    nc.tensor.transpose(
                        pt[:, icc, :], o_sb[:, icc * P : (icc + 1) * P], ident[:]
                    )
                ot = op_.tile([P, IC, P], BF16, tag="ot")
                nc.scalar.copy(ot[:], pt[:])
                ot_state[(g, tcc)] = ot

            def emit_y(g, tcc):
                b, t = g // NT, g % NT
                wo_sb, bo_bc = late_consts["wo_sb"], late_consts["bo_bc"]
                ot = ot_state.pop((g, tcc))
                ysb = yp.tile([P, Q_DIM], F32, tag="y")
                for half in range(2):
                    col = slice(half * 512, (half + 1) * 512)
                    py = ps_y.tile([P, 512], F32, tag="y")
                    for icc in range(IC):
                        nc.tensor.matmul(
                            py[:],
                            lhsT=ot[:, icc, :],
                            rhs=wo_sb[:, icc, col],
                            start=(icc == 0),
                            stop=(icc == IC - 1),
                        )
                    nc.vector.tensor_tensor(
                        out=ysb[:, col],
                        in0=py[:],
                        in1=bo_bc[:, col],
                        op=mybir.AluOpType.add,
                    )
                row0 = b * N + t * TQ + tcc * P
                nc.sync.dma_start(out=y[row0 : row0 + P, :], in_=ysb[:])

            # ---- software-pipelined emission, one-tile skew ----
            # DMA queue order: wq, x(0), wk, wv, ctx(0), wo, bo — the first
            # Q-proj starts as soon as wq+x(0) land; K/V proj slots in right
            # after the first Q chunk.
            emit_dma_x(0)
            emit_kv_weights()
            for g in range(G + 1):
                a_live = g < G
                b_live = g >= 1
                if a_live and g % NT == 0 and g > 0:
                    emit_kv(g // NT)
                # Q chunks lead by one position so every score pair lands
                # right after its qt copy, and all 8 exps retire well
                # before PV(g, tc0) consumes them at the next step's start.
                if a_live:
                    emit_qchunk(g, 0)
                    if g == 0:
                        emit_kv(0)
                        emit_out_weights()
                    emit_qchunk(g, 1)
                if b_live:
                    o_state[(g - 1, 0)] = emit_pv(g - 1, 0)
                for i in range(1, TC):
                    if a_live:
                        if i < TC - 1:
                            emit_qchunk(g, i + 1)
                        emit_score(g, 2 * (i - 1))
                        emit_score(g, 2 * (i - 1) + 1)
                        if i == TC - 1:
                            emit_score(g, 6)
                            emit_score(g, 7)
                    if b_live:
                        o_state[(g - 1, i)] = emit_pv(g - 1, i)
                        emit_tr(g - 1, i - 1)
                        emit_y(g - 1, i - 1)
                if a_live and g + 1 < G:
                    emit_dma_x(g + 1)
                if b_live:
                    emit_tr(g - 1, TC - 1)
                    emit_y(g - 1, TC - 1)
                    # drop per-tile state no longer needed
                    st_t.pop(g - 1, None)
                    qt_t.pop(g - 1, None)
                    xt_t.pop(g - 1, None)
    _split_excess_waits(nc)
    return nc


def prep_in_maps(x, context, Wq, Wk, Wv, Wo, bo):
    bf = ml_dtypes.bfloat16
    # host-side prep: bf16 cast + pre-transpose so contraction dims are
    # contiguous on device partitions
    xT = np.ascontiguousarray(
        np.asarray(x, dtype=np.float32).transpose(0, 2, 1)
    ).astype(bf)
    ctxT = np.ascontiguousarray(
        np.asarray(context, dtype=np.float32).transpose(0, 2, 1)
    ).astype(bf)
    scale = np.float32(1.0 / np.sqrt(D))
    wq_h = (np.asarray(Wq, dtype=np.float32) * scale).astype(bf)
    wk_h = np.asarray(Wk, dtype=np.float32).astype(bf)
    wv_h = np.asarray(Wv, dtype=np.float32).astype(bf)
    wo_h = np.asarray(Wo, dtype=np.float32).astype(bf)
    bo_h = np.ascontiguousarray(
        np.broadcast_to(
            np.asarray(bo, dtype=np.float32).reshape(1, Q_DIM), (P, Q_DIM)
        )
    ).astype(bf)
    in_maps = []
    for c in range(N_CORES):
        in_maps.append(
            {
                "xT": xT[c * B_LOC : (c + 1) * B_LOC],
                "ctxT": ctxT[c * B_LOC : (c + 1) * B_LOC],
                "wq": wq_h,
                "wk": wk_h,
                "wv": wv_h,
                "wo": wo_h,
                "bo": bo_h,
            }
        )
    return in_maps


def kernel(x, context, Wq, Wk, Wv, Wo, bo):
    global LAST_RESULTS
    in_maps = prep_in_maps(x, context, Wq, Wk, Wv, Wo, bo)
    nc = _build_program()
    trace = bool(int(os.environ.get("BASS_KERNEL_TRACE", "0")))
    res = run_bass_kernel_spmd(
        nc, in_maps, core_ids=list(range(N_CORES)), trace=trace
    )
    LAST_RESULTS = res
    out = np.empty((B, N, Q_DIM), dtype=np.float32)
    for c in range(N_CORES):
        out[c * B_LOC : (c + 1) * B_LOC] = res.results[c]["y"].reshape(
            B_LOC, N, Q_DIM
        )
    return out
